# revision 8
# baseline (speedup 1.0000x reference)
"""AttentiveFP forward on 8 Trainium2 NeuronCores.

Sharding: nodes row-sharded 8 ways (data parallel); weights replicated.
The dense per-node matmuls (lin1, per-layer GAT projections, GRU input/hidden
projections, readout projection) run on device via one compiled SPMD Bass
kernel (feature-major tiled matmul, bf16 inputs, fp32 accumulation).
Graph-irregular glue (segment softmax, gathers) runs on host between launches.
"""
import numpy as np
import ml_dtypes
from contextlib import ExitStack

import concourse.bass as bass
import concourse.mybir as mybir
import concourse.tile as tile
from concourse import bass_utils

NCORES = 8
D = 300
KP = 3                 # contraction chunks of 128 (384 padded)
KROWS = KP * 128       # 384
MBLK = 512             # node columns per PSUM block
NBLK = 13              # blocks per core
MCORE = NBLK * MBLK    # 6656 node columns per core
COUT = 1024            # output columns (8 chunks of 128)
GCH = COUT // 128

NEG_TORCH = 0.01
NEG_GAT = 0.2

_BF16 = ml_dtypes.bfloat16
_state = {}


def _build_nc():
    nc = bass.Bass(num_devices=NCORES)
    xt = nc.dram_tensor("xt", [KROWS, MCORE], mybir.dt.bfloat16, kind="ExternalInput")
    ht = nc.dram_tensor("ht", [KROWS, MCORE], mybir.dt.bfloat16, kind="ExternalInput")
    w1 = nc.dram_tensor("w1", [KROWS, COUT], mybir.dt.bfloat16, kind="ExternalInput")
    w2 = nc.dram_tensor("w2", [KROWS, COUT], mybir.dt.bfloat16, kind="ExternalInput")
    o1 = nc.dram_tensor("o1", [COUT, MCORE], mybir.dt.float32, kind="ExternalOutput")
    o2 = nc.dram_tensor("o2", [COUT, MCORE], mybir.dt.float32, kind="ExternalOutput")

    with tile.TileContext(nc) as tc, ExitStack() as ctx:
        wpool = ctx.enter_context(tc.tile_pool(name="w", bufs=1))
        xpool = ctx.enter_context(tc.tile_pool(name="x", bufs=4))
        opool = ctx.enter_context(tc.tile_pool(name="o", bufs=4))
        pspool = ctx.enter_context(tc.tile_pool(name="ps", bufs=4, space="PSUM"))

        wt = {}
        for wi, wdram in ((0, w1), (1, w2)):
            for k in range(KP):
                t = wpool.tile([128, COUT], mybir.dt.bfloat16, name=f"wt{wi}{k}")
                nc.gpsimd.dma_start(out=t[:], in_=wdram[k * 128:(k + 1) * 128, :])
                wt[wi, k] = t

        for nb in range(NBLK):
            cols = slice(nb * MBLK, (nb + 1) * MBLK)
            for wi, src, od in ((0, xt, o1), (1, ht, o2)):
                a = []
                for k in range(KP):
                    ak = xpool.tile([128, MBLK], mybir.dt.bfloat16, name="ak",
                                    tag="ak", bufs=6)
                    nc.sync.dma_start(out=ak[:], in_=src[k * 128:(k + 1) * 128, cols])
                    a.append(ak)
                ob = opool.tile([128, GCH, MBLK], mybir.dt.float32, name="ob",
                                tag="ob", bufs=2)
                for g in range(GCH):
                    ps = pspool.tile([128, MBLK], mybir.dt.float32, space="PSUM")
                    for k in range(KP):
                        nc.tensor.matmul(
                            out=ps[:],
                            lhsT=wt[wi, k][:, g * 128:(g + 1) * 128],
                            rhs=a[k][:],
                            start=(k == 0), stop=(k == KP - 1),
                        )
                    nc.scalar.copy(out=ob[:, g, :], in_=ps[:])
                # single big DMA: [128, g, c] -> od rows g*128+p, cols
                od_view = bass.AP(
                    tensor=od[:].tensor,
                    offset=nb * MBLK,
                    ap=[[MCORE, 128], [128 * MCORE, GCH], [1, MBLK]],
                )
                nc.scalar.dma_start(out=od_view, in_=ob[:])

    # Walrus limits DMA pseudo-instructions to 2 sync waits. Move any excess
    # onto a same-engine NoOp placed immediately before (engine queues are
    # in-order, so an earlier wait is equivalent).
    for f in nc.m.functions:
        for bb in f.blocks:
            newlist = []
            for inst in bb.instructions:
                si = inst.sync_info
                if si is not None and si.on_wait and len(si.on_wait) > 1:
                    extra = list(si.on_wait[:-1])
                    keep = list(si.on_wait[-1:])
                    for wi_, w in enumerate(extra):
                        nop = mybir.InstNoOp(
                            name=f"{inst.name}-waitnop{wi_}", ins=[], outs=[],
                            sync_info=mybir.SyncInfo(on_wait=[w], on_update=[]),
                        )
                        nop.engine = inst.engine
                        newlist.append(nop)
                    si.on_wait = keep
                newlist.append(inst)
            bb.instructions = newlist
    return nc


def _get_nc():
    if "nc" not in _state:
        _state["nc"] = _build_nc()
    return _state["nc"]


def _pad_w(w):
    """[300, C] -> [384, 1024] bf16"""
    out = np.zeros((KROWS, COUT), dtype=_BF16)
    out[: w.shape[0], : w.shape[1]] = w.astype(_BF16)
    return out


_zw = None


def dev_mm(A, W1, B=None, W2=None):
    """Returns (A @ W1, B @ W2) computed on the 8-core device kernel.
    A, B: [N, 300] float32 (N <= 8*MCORE). W1, W2: [300, C<=1024]."""
    global _zw
    nc = _get_nc()
    N = A.shape[0]
    if _zw is None:
        _zw = np.zeros((KROWS, COUT), dtype=_BF16)
    w1 = _pad_w(W1)
    w2 = _pad_w(W2) if W2 is not None else _zw

    def shard(X):
        xt = np.zeros((NCORES, KROWS, MCORE), dtype=_BF16)
        if X is not None:
            Xt = np.ascontiguousarray(X.T.astype(_BF16))  # [300, N]
            for c in range(NCORES):
                lo = c * MCORE
                hi = min(N, lo + MCORE)
                if hi > lo:
                    xt[c, :D, : hi - lo] = Xt[:, lo:hi]
        return xt

    xts = shard(A)
    hts = shard(B)
    in_maps = [dict(xt=xts[c], ht=hts[c], w1=w1, w2=w2) for c in range(NCORES)]
    res = bass_utils.run_bass_kernel_spmd(nc, in_maps, core_ids=list(range(NCORES)))
    _state.setdefault("launches", 0)
    _state["launches"] += 1

    def gather(key, C):
        out = np.empty((N, C), dtype=np.float32)
        for c in range(NCORES):
            lo = c * MCORE
            hi = min(N, lo + MCORE)
            if hi > lo:
                out[lo:hi] = res.results[c][key][:C, : hi - lo].T
        return out

    oA = gather("o1", W1.shape[1])
    oB = gather("o2", W2.shape[1]) if W2 is not None else None
    return oA, oB


# ---------------- host-side numerics ----------------

def _lrelu(x, a):
    return np.where(x > 0, x, a * x)


def _elu(x):
    return np.where(x > 0, x, np.expm1(np.minimum(x, 0.0)))


def _sigmoid(x):
    return 1.0 / (1.0 + np.exp(-x))


def _seg_softmax(a, idx, n):
    m = np.full(n, -np.inf, dtype=np.float32)
    np.maximum.at(m, idx, a)
    e = np.exp(a - np.where(np.isfinite(m[idx]), m[idx], 0.0))
    s = np.zeros(n, dtype=np.float32)
    np.add.at(s, idx, e)
    return e / (s[idx] + 1e-16)


def _seg_sum(vals, idx, n):
    out = np.zeros((n, vals.shape[1]), dtype=np.float32)
    np.add.at(out, idx, vals)
    return out


def _gru_gates(gi, gh, h):
    ir, iz, in_ = gi[:, :D], gi[:, D:2 * D], gi[:, 2 * D:3 * D]
    hr, hz, hn = gh[:, :D], gh[:, D:2 * D], gh[:, 2 * D:3 * D]
    r = _sigmoid(ir + hr)
    z = _sigmoid(iz + hz)
    n = np.tanh(in_ + r * hn)
    return (1.0 - z) * n + z * h


def kernel(x, edge_index, edge_attr, batch,
           emb_atom, emb_chir, lin1_W, lin1_b,
           ge_e1, ge_e2, ge_att_l, ge_att_r, ge_lin1_W, ge_lin2_W, ge_bias,
           gat_W, gat_att_s, gat_att_d, gat_b,
           gru_wih, gru_whh, gru_bih, gru_bhh,
           mol_Ws, mol_Wd, mol_att_s, mol_att_d, mol_b,
           mgru_wih, mgru_whh, mgru_bih, mgru_bhh,
           lin2_W, lin2_b):
    f32 = np.float32
    x = np.asarray(x)
    edge_index = np.asarray(edge_index).astype(np.int64)
    edge_attr = np.asarray(edge_attr)
    batch = np.asarray(batch).astype(np.int64)
    args = {k: np.asarray(v, dtype=f32) for k, v in dict(
        emb_atom=emb_atom, emb_chir=emb_chir, lin1_W=lin1_W, lin1_b=lin1_b,
        ge_e1=ge_e1, ge_e2=ge_e2, ge_att_l=ge_att_l, ge_att_r=ge_att_r,
        ge_lin1_W=ge_lin1_W, ge_lin2_W=ge_lin2_W, ge_bias=ge_bias,
        gat_W=gat_W, gat_att_s=gat_att_s, gat_att_d=gat_att_d, gat_b=gat_b,
        gru_wih=gru_wih, gru_whh=gru_whh, gru_bih=gru_bih, gru_bhh=gru_bhh,
        mol_Ws=mol_Ws, mol_Wd=mol_Wd, mol_att_s=mol_att_s, mol_att_d=mol_att_d,
        mol_b=mol_b, mgru_wih=mgru_wih, mgru_whh=mgru_whh, mgru_bih=mgru_bih,
        mgru_bhh=mgru_bhh, lin2_W=lin2_W, lin2_b=lin2_b).items()}
    g = args
    N = x.shape[0]
    G = 2000

    # node embedding + lin1 (device matmul)
    emb = g["emb_atom"][x[:, 0]] + g["emb_chir"][x[:, 1]]
    o, _ = dev_mm(emb, g["lin1_W"].T)
    xf = _lrelu(o + g["lin1_b"], NEG_TORCH)

    # --- layer 0: GATEConv with self loops + edge embeddings ---
    loop = np.arange(N)
    src = np.concatenate([edge_index[0], loop])
    dst = np.concatenate([edge_index[1], loop])
    bt = np.concatenate([edge_attr[:, 0], np.full(N, 4, edge_attr.dtype)])
    bd = np.concatenate([edge_attr[:, 1], np.zeros(N, edge_attr.dtype)])
    eemb = g["ge_e1"][bt] + g["ge_e2"][bd]

    W_a = g["ge_lin1_W"][:, :D]      # acts on xf[src]
    W_b = g["ge_lin1_W"][:, D:]      # acts on eemb
    # u1 = xf @ W_a.T, r_att = xf @ ge_att_r  (device, packed)
    o, gh = dev_mm(xf, np.concatenate([W_a.T, g["ge_att_r"][:, None]], 1),
                   xf, g["gru_whh"][0].T)
    u1, r_att = o[:, :D], o[:, D]
    # eemb @ W_b.T has only a few distinct rows; tiny host matmul
    t_small = eemb @ W_b.T
    xj = _lrelu(u1[src] + t_small, NEG_TORCH)
    alpha = _lrelu(xj @ g["ge_att_l"] + r_att[dst], NEG_TORCH)
    alpha = _seg_softmax(alpha, dst, N)
    agg = _seg_sum(xj * alpha[:, None], dst, N)
    o, _ = dev_mm(agg, g["ge_lin2_W"].T)
    h = _elu(o + g["ge_bias"])
    gi, _ = dev_mm(h, g["gru_wih"][0].T)
    xf = np.maximum(_gru_gates(gi + g["gru_bih"][0], gh + g["gru_bhh"][0], xf), 0.0)

    # --- layers 1..4: GATConv ---
    s, d2 = edge_index[0], edge_index[1]
    for l in range(4):
        Wp = np.concatenate([g["gat_W"][l].T,
                             (g["gat_W"][l].T @ g["gat_att_s"][l])[:, None],
                             (g["gat_W"][l].T @ g["gat_att_d"][l])[:, None]], 1)
        o, gh = dev_mm(xf, Wp, xf, g["gru_whh"][l + 1].T)
        xw, a_s, a_d = o[:, :D], o[:, D], o[:, D + 1]
        alpha = _lrelu(a_s[s] + a_d[d2], NEG_GAT)
        alpha = _seg_softmax(alpha, d2, N)
        h = _seg_sum(xw[s] * alpha[:, None], d2, N) + g["gat_b"][l]
        gi, _ = dev_mm(_elu(h), g["gru_wih"][l + 1].T)
        xf = np.maximum(_gru_gates(gi + g["gru_bih"][l + 1],
                                   gh + g["gru_bhh"][l + 1], xf), 0.0)

    # --- attentive readout ---
    out = np.maximum(_seg_sum(xf, batch, G), 0.0)
    Wp = np.concatenate([g["mol_Ws"].T,
                         (g["mol_Ws"].T @ g["mol_att_s"])[:, None]], 1)
    o, _ = dev_mm(xf, Wp)
    xs, xs_att = o[:, :D], o[:, D]
    for _ in range(3):
        xd_att = (out @ g["mol_Wd"].T) @ g["mol_att_d"]
        alpha = _lrelu(xs_att + xd_att[batch], NEG_GAT)
        alpha = _seg_softmax(alpha, batch, G)
        h = _seg_sum(xs * alpha[:, None], batch, G) + g["mol_b"]
        gi = _elu(h) @ g["mgru_wih"].T + g["mgru_bih"]
        ghm = out @ g["mgru_whh"].T + g["mgru_bhh"]
        out = np.maximum(_gru_gates(gi, ghm, out), 0.0)

    pred = out @ g["lin2_W"].T + g["lin2_b"]
    return (out.astype(f32), pred.astype(f32))


# revision 9
# speedup vs baseline: 1.2148x; 1.2148x over previous
"""AttentiveFP forward on 8 Trainium2 NeuronCores.

Sharding: nodes row-sharded 8 ways (data parallel); weights replicated.
The dense per-node matmuls (lin1, per-layer GAT projections, GRU input/hidden
projections, readout projection) run on device via one compiled SPMD Bass
kernel (feature-major tiled matmul, bf16 inputs, fp32 accumulation).
Graph-irregular glue (segment softmax, gathers) runs on host between launches.
"""
import numpy as np
import ml_dtypes
from contextlib import ExitStack

import concourse.bass as bass
import concourse.mybir as mybir
import concourse.tile as tile
from concourse import bass_utils

NCORES = 8
D = 300
KCH = [(0, 128), (128, 128), (256, 48)]   # contraction chunks (304 rows)
KP = 3
KROWS = 304
MBLK = 512             # node columns per PSUM block
NBLK = 13              # blocks per core
MCORE = NBLK * MBLK    # 6656 node columns per core
COUT = 1024            # output columns (8 chunks of 128)
GCH = COUT // 128

NEG_TORCH = 0.01
NEG_GAT = 0.2

_BF16 = ml_dtypes.bfloat16
_state = {}


def _build_nc():
    nc = bass.Bass(num_devices=NCORES)
    xt = nc.dram_tensor("xt", [KROWS, MCORE], mybir.dt.bfloat16, kind="ExternalInput")
    ht = nc.dram_tensor("ht", [KROWS, MCORE], mybir.dt.bfloat16, kind="ExternalInput")
    w1 = nc.dram_tensor("w1", [KROWS, COUT], mybir.dt.bfloat16, kind="ExternalInput")
    w2 = nc.dram_tensor("w2", [KROWS, COUT], mybir.dt.bfloat16, kind="ExternalInput")
    o1 = nc.dram_tensor("o1", [COUT, MCORE], mybir.dt.float32, kind="ExternalOutput")
    o2 = nc.dram_tensor("o2", [COUT, MCORE], mybir.dt.float32, kind="ExternalOutput")

    with tile.TileContext(nc) as tc, ExitStack() as ctx:
        wpool = ctx.enter_context(tc.tile_pool(name="w", bufs=1))
        xpool = ctx.enter_context(tc.tile_pool(name="x", bufs=4))
        opool = ctx.enter_context(tc.tile_pool(name="o", bufs=4))
        pspool = ctx.enter_context(tc.tile_pool(name="ps", bufs=4, space="PSUM"))

        wt = {}
        for wi, wdram in ((0, w1), (1, w2)):
            for k in range(KP):
                ko, kn = KCH[k]
                t = wpool.tile([128, COUT], mybir.dt.bfloat16, name=f"wt{wi}{k}")
                nc.gpsimd.dma_start(out=t[:kn, :], in_=wdram[ko:ko + kn, :])
                wt[wi, k] = t

        for nb in range(NBLK):
            cols = slice(nb * MBLK, (nb + 1) * MBLK)
            for wi, src, od in ((0, xt, o1), (1, ht, o2)):
                a = []
                for k in range(KP):
                    ko, kn = KCH[k]
                    ak = xpool.tile([128, MBLK], mybir.dt.bfloat16, name="ak",
                                    tag="ak", bufs=6)
                    nc.sync.dma_start(out=ak[:kn, :], in_=src[ko:ko + kn, cols])
                    a.append(ak)
                ob = opool.tile([128, GCH, MBLK], mybir.dt.float32, name="ob",
                                tag="ob", bufs=2)
                for g in range(GCH):
                    ps = pspool.tile([128, MBLK], mybir.dt.float32, space="PSUM")
                    for k in range(KP):
                        kn = KCH[k][1]
                        nc.tensor.matmul(
                            out=ps[:],
                            lhsT=wt[wi, k][:kn, g * 128:(g + 1) * 128],
                            rhs=a[k][:kn, :],
                            start=(k == 0), stop=(k == KP - 1),
                        )
                    nc.scalar.copy(out=ob[:, g, :], in_=ps[:])
                # single big DMA: [128, g, c] -> od rows g*128+p, cols
                od_view = bass.AP(
                    tensor=od[:].tensor,
                    offset=nb * MBLK,
                    ap=[[MCORE, 128], [128 * MCORE, GCH], [1, MBLK]],
                )
                nc.scalar.dma_start(out=od_view, in_=ob[:])

    # Walrus limits DMA pseudo-instructions to 2 sync waits. Move any excess
    # onto a same-engine NoOp placed immediately before (engine queues are
    # in-order, so an earlier wait is equivalent).
    for f in nc.m.functions:
        for bb in f.blocks:
            newlist = []
            for inst in bb.instructions:
                si = inst.sync_info
                if si is not None and si.on_wait and len(si.on_wait) > 1:
                    extra = list(si.on_wait[:-1])
                    keep = list(si.on_wait[-1:])
                    for wi_, w in enumerate(extra):
                        nop = mybir.InstNoOp(
                            name=f"{inst.name}-waitnop{wi_}", ins=[], outs=[],
                            sync_info=mybir.SyncInfo(on_wait=[w], on_update=[]),
                        )
                        nop.engine = inst.engine
                        newlist.append(nop)
                    si.on_wait = keep
                newlist.append(inst)
            bb.instructions = newlist
    return nc


def _get_nc():
    if "nc" not in _state:
        _state["nc"] = _build_nc()
    return _state["nc"]


def _pad_w(w):
    """[300, C] -> [384, 1024] bf16"""
    out = np.zeros((KROWS, COUT), dtype=_BF16)
    out[: w.shape[0], : w.shape[1]] = w.astype(_BF16)
    return out


_zw = None


def dev_mm(A, W1, B=None, W2=None):
    """Returns (A @ W1, B @ W2) computed on the 8-core device kernel.
    A, B: [N, 300] float32 (N <= 8*MCORE). W1, W2: [300, C<=1024]."""
    global _zw
    nc = _get_nc()
    N = A.shape[0]
    if _zw is None:
        _zw = np.zeros((KROWS, COUT), dtype=_BF16)
    w1 = _pad_w(W1)
    w2 = _pad_w(W2) if W2 is not None else _zw

    def shard(X):
        xt = np.zeros((NCORES, KROWS, MCORE), dtype=_BF16)
        if X is not None:
            Xt = np.ascontiguousarray(X.T.astype(_BF16))  # [300, N]
            for c in range(NCORES):
                lo = c * MCORE
                hi = min(N, lo + MCORE)
                if hi > lo:
                    xt[c, :D, : hi - lo] = Xt[:, lo:hi]
        return xt

    xts = shard(A)
    hts = shard(B)
    in_maps = [dict(xt=xts[c], ht=hts[c], w1=w1, w2=w2) for c in range(NCORES)]
    res = bass_utils.run_bass_kernel_spmd(nc, in_maps, core_ids=list(range(NCORES)))
    _state.setdefault("launches", 0)
    _state["launches"] += 1

    def gather(key, C):
        out = np.empty((N, C), dtype=np.float32)
        for c in range(NCORES):
            lo = c * MCORE
            hi = min(N, lo + MCORE)
            if hi > lo:
                out[lo:hi] = res.results[c][key][:C, : hi - lo].T
        return out

    oA = gather("o1", W1.shape[1])
    oB = gather("o2", W2.shape[1]) if W2 is not None else None
    return oA, oB


# ---------------- host-side numerics ----------------

def _lrelu(x, a):
    return np.where(x > 0, x, a * x)


def _elu(x):
    return np.where(x > 0, x, np.expm1(np.minimum(x, 0.0)))


def _sigmoid(x):
    return 1.0 / (1.0 + np.exp(-x))


def _seg_softmax(a, idx, n):
    m = np.full(n, -np.inf, dtype=np.float32)
    np.maximum.at(m, idx, a)
    e = np.exp(a - np.where(np.isfinite(m[idx]), m[idx], 0.0))
    s = np.zeros(n, dtype=np.float32)
    np.add.at(s, idx, e)
    return e / (s[idx] + 1e-16)


def _seg_sum(vals, idx, n):
    out = np.zeros((n, vals.shape[1]), dtype=np.float32)
    np.add.at(out, idx, vals)
    return out


def _gru_gates(gi, gh, h):
    ir, iz, in_ = gi[:, :D], gi[:, D:2 * D], gi[:, 2 * D:3 * D]
    hr, hz, hn = gh[:, :D], gh[:, D:2 * D], gh[:, 2 * D:3 * D]
    r = _sigmoid(ir + hr)
    z = _sigmoid(iz + hz)
    n = np.tanh(in_ + r * hn)
    return (1.0 - z) * n + z * h


def kernel(x, edge_index, edge_attr, batch,
           emb_atom, emb_chir, lin1_W, lin1_b,
           ge_e1, ge_e2, ge_att_l, ge_att_r, ge_lin1_W, ge_lin2_W, ge_bias,
           gat_W, gat_att_s, gat_att_d, gat_b,
           gru_wih, gru_whh, gru_bih, gru_bhh,
           mol_Ws, mol_Wd, mol_att_s, mol_att_d, mol_b,
           mgru_wih, mgru_whh, mgru_bih, mgru_bhh,
           lin2_W, lin2_b):
    f32 = np.float32
    x = np.asarray(x)
    edge_index = np.asarray(edge_index).astype(np.int64)
    edge_attr = np.asarray(edge_attr)
    batch = np.asarray(batch).astype(np.int64)
    args = {k: np.asarray(v, dtype=f32) for k, v in dict(
        emb_atom=emb_atom, emb_chir=emb_chir, lin1_W=lin1_W, lin1_b=lin1_b,
        ge_e1=ge_e1, ge_e2=ge_e2, ge_att_l=ge_att_l, ge_att_r=ge_att_r,
        ge_lin1_W=ge_lin1_W, ge_lin2_W=ge_lin2_W, ge_bias=ge_bias,
        gat_W=gat_W, gat_att_s=gat_att_s, gat_att_d=gat_att_d, gat_b=gat_b,
        gru_wih=gru_wih, gru_whh=gru_whh, gru_bih=gru_bih, gru_bhh=gru_bhh,
        mol_Ws=mol_Ws, mol_Wd=mol_Wd, mol_att_s=mol_att_s, mol_att_d=mol_att_d,
        mol_b=mol_b, mgru_wih=mgru_wih, mgru_whh=mgru_whh, mgru_bih=mgru_bih,
        mgru_bhh=mgru_bhh, lin2_W=lin2_W, lin2_b=lin2_b).items()}
    g = args
    N = x.shape[0]
    G = 2000

    # node embedding + lin1 (device matmul)
    emb = g["emb_atom"][x[:, 0]] + g["emb_chir"][x[:, 1]]
    o, _ = dev_mm(emb, g["lin1_W"].T)
    xf = _lrelu(o + g["lin1_b"], NEG_TORCH)

    # --- layer 0: GATEConv with self loops + edge embeddings ---
    loop = np.arange(N)
    src = np.concatenate([edge_index[0], loop])
    dst = np.concatenate([edge_index[1], loop])
    bt = np.concatenate([edge_attr[:, 0], np.full(N, 4, edge_attr.dtype)])
    bd = np.concatenate([edge_attr[:, 1], np.zeros(N, edge_attr.dtype)])
    eemb = g["ge_e1"][bt] + g["ge_e2"][bd]

    W_a = g["ge_lin1_W"][:, :D]      # acts on xf[src]
    W_b = g["ge_lin1_W"][:, D:]      # acts on eemb
    # u1 = xf @ W_a.T, r_att = xf @ ge_att_r  (device, packed)
    o, gh = dev_mm(xf, np.concatenate([W_a.T, g["ge_att_r"][:, None]], 1),
                   xf, g["gru_whh"][0].T)
    u1, r_att = o[:, :D], o[:, D]
    # eemb @ W_b.T has only a few distinct rows; tiny host matmul
    t_small = eemb @ W_b.T
    xj = _lrelu(u1[src] + t_small, NEG_TORCH)
    alpha = _lrelu(xj @ g["ge_att_l"] + r_att[dst], NEG_TORCH)
    alpha = _seg_softmax(alpha, dst, N)
    agg = _seg_sum(xj * alpha[:, None], dst, N)
    o, _ = dev_mm(agg, g["ge_lin2_W"].T)
    h = _elu(o + g["ge_bias"])
    gi, _ = dev_mm(h, g["gru_wih"][0].T)
    xf = np.maximum(_gru_gates(gi + g["gru_bih"][0], gh + g["gru_bhh"][0], xf), 0.0)

    # --- layers 1..4: GATConv ---
    s, d2 = edge_index[0], edge_index[1]
    for l in range(4):
        Wp = np.concatenate([g["gat_W"][l].T,
                             (g["gat_W"][l].T @ g["gat_att_s"][l])[:, None],
                             (g["gat_W"][l].T @ g["gat_att_d"][l])[:, None]], 1)
        o, gh = dev_mm(xf, Wp, xf, g["gru_whh"][l + 1].T)
        xw, a_s, a_d = o[:, :D], o[:, D], o[:, D + 1]
        alpha = _lrelu(a_s[s] + a_d[d2], NEG_GAT)
        alpha = _seg_softmax(alpha, d2, N)
        h = _seg_sum(xw[s] * alpha[:, None], d2, N) + g["gat_b"][l]
        gi, _ = dev_mm(_elu(h), g["gru_wih"][l + 1].T)
        xf = np.maximum(_gru_gates(gi + g["gru_bih"][l + 1],
                                   gh + g["gru_bhh"][l + 1], xf), 0.0)

    # --- attentive readout ---
    out = np.maximum(_seg_sum(xf, batch, G), 0.0)
    Wp = np.concatenate([g["mol_Ws"].T,
                         (g["mol_Ws"].T @ g["mol_att_s"])[:, None]], 1)
    o, _ = dev_mm(xf, Wp)
    xs, xs_att = o[:, :D], o[:, D]
    for _ in range(3):
        xd_att = (out @ g["mol_Wd"].T) @ g["mol_att_d"]
        alpha = _lrelu(xs_att + xd_att[batch], NEG_GAT)
        alpha = _seg_softmax(alpha, batch, G)
        h = _seg_sum(xs * alpha[:, None], batch, G) + g["mol_b"]
        gi = _elu(h) @ g["mgru_wih"].T + g["mgru_bih"]
        ghm = out @ g["mgru_whh"].T + g["mgru_bhh"]
        out = np.maximum(_gru_gates(gi, ghm, out), 0.0)

    pred = out @ g["lin2_W"].T + g["lin2_b"]
    return (out.astype(f32), pred.astype(f32))


# revision 10
# speedup vs baseline: 1.5735x; 1.2953x over previous
"""AttentiveFP forward on 8 Trainium2 NeuronCores.

Sharding: nodes row-sharded 8 ways (data parallel); weights replicated.
The dense per-node matmuls (lin1, per-layer GAT projections, GRU input/hidden
projections, readout projection) run on device via one compiled SPMD Bass
kernel (feature-major tiled matmul, bf16 inputs, fp32 accumulation).
Graph-irregular glue (segment softmax, gathers) runs on host between launches.
"""
import numpy as np
import ml_dtypes
from contextlib import ExitStack

import concourse.bass as bass
import concourse.mybir as mybir
import concourse.tile as tile
from concourse import bass_utils

NCORES = 8
D = 300
KCH = [(0, 128), (128, 128), (256, 48)]   # contraction chunks (304 rows)
KP = 3
KROWS = 304
MBLK = 512             # node columns per PSUM block
NBLK = 13              # blocks per core
MCORE = NBLK * MBLK    # 6656 node columns per core
COUT = 1024            # output columns (8 chunks of 128)
GCH = COUT // 128

NEG_TORCH = 0.01
NEG_GAT = 0.2

_BF16 = ml_dtypes.bfloat16
_state = {}


def _build_nc(dual=True):
    nc = bass.Bass(num_devices=NCORES)
    xt = nc.dram_tensor("xt", [KROWS, MCORE], mybir.dt.bfloat16, kind="ExternalInput")
    w1 = nc.dram_tensor("w1", [KROWS, COUT], mybir.dt.bfloat16, kind="ExternalInput")
    o1 = nc.dram_tensor("o1", [COUT, MCORE], mybir.dt.float32, kind="ExternalOutput")
    if dual:
        ht = nc.dram_tensor("ht", [KROWS, MCORE], mybir.dt.bfloat16, kind="ExternalInput")
        w2 = nc.dram_tensor("w2", [KROWS, COUT], mybir.dt.bfloat16, kind="ExternalInput")
        o2 = nc.dram_tensor("o2", [COUT, MCORE], mybir.dt.float32, kind="ExternalOutput")

    with tile.TileContext(nc) as tc, ExitStack() as ctx:
        wpool = ctx.enter_context(tc.tile_pool(name="w", bufs=1))
        xpool = ctx.enter_context(tc.tile_pool(name="x", bufs=4))
        opool = ctx.enter_context(tc.tile_pool(name="o", bufs=4))
        pspool = ctx.enter_context(tc.tile_pool(name="ps", bufs=4, space="PSUM"))

        wt = {}
        wsrcs = ((0, w1), (1, w2)) if dual else ((0, w1),)
        for wi, wdram in wsrcs:
            for k in range(KP):
                ko, kn = KCH[k]
                t = wpool.tile([128, COUT], mybir.dt.bfloat16, name=f"wt{wi}{k}")
                nc.gpsimd.dma_start(out=t[:kn, :], in_=wdram[ko:ko + kn, :])
                wt[wi, k] = t

        for nb in range(NBLK):
            cols = slice(nb * MBLK, (nb + 1) * MBLK)
            srcs = ((0, xt, o1), (1, ht, o2)) if dual else ((0, xt, o1),)
            for wi, src, od in srcs:
                a = []
                for k in range(KP):
                    ko, kn = KCH[k]
                    ak = xpool.tile([128, MBLK], mybir.dt.bfloat16, name="ak",
                                    tag="ak", bufs=6)
                    nc.sync.dma_start(out=ak[:kn, :], in_=src[ko:ko + kn, cols])
                    a.append(ak)
                ob = opool.tile([128, GCH, MBLK], mybir.dt.float32, name="ob",
                                tag="ob", bufs=2)
                for g in range(GCH):
                    ps = pspool.tile([128, MBLK], mybir.dt.float32, space="PSUM")
                    for k in range(KP):
                        kn = KCH[k][1]
                        nc.tensor.matmul(
                            out=ps[:],
                            lhsT=wt[wi, k][:kn, g * 128:(g + 1) * 128],
                            rhs=a[k][:kn, :],
                            start=(k == 0), stop=(k == KP - 1),
                        )
                    nc.scalar.copy(out=ob[:, g, :], in_=ps[:])
                # single big DMA: [128, g, c] -> od rows g*128+p, cols
                od_view = bass.AP(
                    tensor=od[:].tensor,
                    offset=nb * MBLK,
                    ap=[[MCORE, 128], [128 * MCORE, GCH], [1, MBLK]],
                )
                nc.scalar.dma_start(out=od_view, in_=ob[:])

    # Walrus limits DMA pseudo-instructions to 2 sync waits. Move any excess
    # onto a same-engine NoOp placed immediately before (engine queues are
    # in-order, so an earlier wait is equivalent).
    for f in nc.m.functions:
        for bb in f.blocks:
            newlist = []
            for inst in bb.instructions:
                si = inst.sync_info
                if si is not None and si.on_wait and len(si.on_wait) > 1:
                    extra = list(si.on_wait[:-1])
                    keep = list(si.on_wait[-1:])
                    for wi_, w in enumerate(extra):
                        nop = mybir.InstNoOp(
                            name=f"{inst.name}-waitnop{wi_}", ins=[], outs=[],
                            sync_info=mybir.SyncInfo(on_wait=[w], on_update=[]),
                        )
                        nop.engine = inst.engine
                        newlist.append(nop)
                    si.on_wait = keep
                newlist.append(inst)
            bb.instructions = newlist
    return nc


def _get_nc(dual=True):
    key = "nc_dual" if dual else "nc_single"
    if key not in _state:
        _state[key] = _build_nc(dual)
    return _state[key]


def _pad_w(w):
    """[300, C] -> [384, 1024] bf16"""
    out = np.zeros((KROWS, COUT), dtype=_BF16)
    out[: w.shape[0], : w.shape[1]] = w.astype(_BF16)
    return out


_zw = None


def dev_mm(A, W1, B=None, W2=None):
    """Returns (A @ W1, B @ W2) computed on the 8-core device kernel.
    A, B: [N, 300] float32 (N <= 8*MCORE). W1, W2: [300, C<=1024]."""
    dual = W2 is not None
    nc = _get_nc(dual)
    N = A.shape[0]
    w1 = _pad_w(W1)
    w2 = _pad_w(W2) if dual else None

    def shard(X):
        xt = np.zeros((NCORES, KROWS, MCORE), dtype=_BF16)
        if X is not None:
            Xt = np.ascontiguousarray(X.T.astype(_BF16))  # [300, N]
            for c in range(NCORES):
                lo = c * MCORE
                hi = min(N, lo + MCORE)
                if hi > lo:
                    xt[c, :D, : hi - lo] = Xt[:, lo:hi]
        return xt

    xts = shard(A)
    if dual:
        hts = shard(B)
        in_maps = [dict(xt=xts[c], ht=hts[c], w1=w1, w2=w2) for c in range(NCORES)]
    else:
        in_maps = [dict(xt=xts[c], w1=w1) for c in range(NCORES)]
    res = bass_utils.run_bass_kernel_spmd(nc, in_maps, core_ids=list(range(NCORES)))
    _state.setdefault("launches", 0)
    _state["launches"] += 1

    def gather(key, C):
        out = np.empty((N, C), dtype=np.float32)
        for c in range(NCORES):
            lo = c * MCORE
            hi = min(N, lo + MCORE)
            if hi > lo:
                out[lo:hi] = res.results[c][key][:C, : hi - lo].T
        return out

    oA = gather("o1", W1.shape[1])
    oB = gather("o2", W2.shape[1]) if dual else None
    return oA, oB


# ---------------- host-side numerics ----------------

def _lrelu(x, a):
    return np.where(x > 0, x, a * x)


def _elu(x):
    return np.where(x > 0, x, np.expm1(np.minimum(x, 0.0)))


def _sigmoid(x):
    return 1.0 / (1.0 + np.exp(-x))


def _seg_softmax(a, idx, n):
    m = np.full(n, -np.inf, dtype=np.float32)
    np.maximum.at(m, idx, a)
    e = np.exp(a - np.where(np.isfinite(m[idx]), m[idx], 0.0))
    s = np.zeros(n, dtype=np.float32)
    np.add.at(s, idx, e)
    return e / (s[idx] + 1e-16)


def _seg_sum(vals, idx, n):
    out = np.zeros((n, vals.shape[1]), dtype=np.float32)
    np.add.at(out, idx, vals)
    return out


def _gru_gates(gi, gh, h):
    ir, iz, in_ = gi[:, :D], gi[:, D:2 * D], gi[:, 2 * D:3 * D]
    hr, hz, hn = gh[:, :D], gh[:, D:2 * D], gh[:, 2 * D:3 * D]
    r = _sigmoid(ir + hr)
    z = _sigmoid(iz + hz)
    n = np.tanh(in_ + r * hn)
    return (1.0 - z) * n + z * h


def kernel(x, edge_index, edge_attr, batch,
           emb_atom, emb_chir, lin1_W, lin1_b,
           ge_e1, ge_e2, ge_att_l, ge_att_r, ge_lin1_W, ge_lin2_W, ge_bias,
           gat_W, gat_att_s, gat_att_d, gat_b,
           gru_wih, gru_whh, gru_bih, gru_bhh,
           mol_Ws, mol_Wd, mol_att_s, mol_att_d, mol_b,
           mgru_wih, mgru_whh, mgru_bih, mgru_bhh,
           lin2_W, lin2_b):
    f32 = np.float32
    x = np.asarray(x)
    edge_index = np.asarray(edge_index).astype(np.int64)
    edge_attr = np.asarray(edge_attr)
    batch = np.asarray(batch).astype(np.int64)
    args = {k: np.asarray(v, dtype=f32) for k, v in dict(
        emb_atom=emb_atom, emb_chir=emb_chir, lin1_W=lin1_W, lin1_b=lin1_b,
        ge_e1=ge_e1, ge_e2=ge_e2, ge_att_l=ge_att_l, ge_att_r=ge_att_r,
        ge_lin1_W=ge_lin1_W, ge_lin2_W=ge_lin2_W, ge_bias=ge_bias,
        gat_W=gat_W, gat_att_s=gat_att_s, gat_att_d=gat_att_d, gat_b=gat_b,
        gru_wih=gru_wih, gru_whh=gru_whh, gru_bih=gru_bih, gru_bhh=gru_bhh,
        mol_Ws=mol_Ws, mol_Wd=mol_Wd, mol_att_s=mol_att_s, mol_att_d=mol_att_d,
        mol_b=mol_b, mgru_wih=mgru_wih, mgru_whh=mgru_whh, mgru_bih=mgru_bih,
        mgru_bhh=mgru_bhh, lin2_W=lin2_W, lin2_b=lin2_b).items()}
    g = args
    N = x.shape[0]
    G = 2000

    # node embedding + lin1 (device matmul)
    emb = g["emb_atom"][x[:, 0]] + g["emb_chir"][x[:, 1]]
    o, _ = dev_mm(emb, g["lin1_W"].T)
    xf = _lrelu(o + g["lin1_b"], NEG_TORCH)

    # --- layer 0: GATEConv with self loops + edge embeddings ---
    loop = np.arange(N)
    src = np.concatenate([edge_index[0], loop])
    dst = np.concatenate([edge_index[1], loop])
    bt = np.concatenate([edge_attr[:, 0], np.full(N, 4, edge_attr.dtype)])
    bd = np.concatenate([edge_attr[:, 1], np.zeros(N, edge_attr.dtype)])
    eemb = g["ge_e1"][bt] + g["ge_e2"][bd]

    W_a = g["ge_lin1_W"][:, :D]      # acts on xf[src]
    W_b = g["ge_lin1_W"][:, D:]      # acts on eemb
    # u1 = xf @ W_a.T, r_att = xf @ ge_att_r  (device, packed)
    o, gh = dev_mm(xf, np.concatenate([W_a.T, g["ge_att_r"][:, None]], 1),
                   xf, g["gru_whh"][0].T)
    u1, r_att = o[:, :D], o[:, D]
    # eemb @ W_b.T has only a few distinct rows; tiny host matmul
    t_small = eemb @ W_b.T
    xj = _lrelu(u1[src] + t_small, NEG_TORCH)
    alpha = _lrelu(xj @ g["ge_att_l"] + r_att[dst], NEG_TORCH)
    alpha = _seg_softmax(alpha, dst, N)
    agg = _seg_sum(xj * alpha[:, None], dst, N)
    o, _ = dev_mm(agg, g["ge_lin2_W"].T)
    h = _elu(o + g["ge_bias"])
    gi, _ = dev_mm(h, g["gru_wih"][0].T)
    xf = np.maximum(_gru_gates(gi + g["gru_bih"][0], gh + g["gru_bhh"][0], xf), 0.0)

    # --- layers 1..4: GATConv ---
    s, d2 = edge_index[0], edge_index[1]
    for l in range(4):
        Wp = np.concatenate([g["gat_W"][l].T,
                             (g["gat_W"][l].T @ g["gat_att_s"][l])[:, None],
                             (g["gat_W"][l].T @ g["gat_att_d"][l])[:, None]], 1)
        o, gh = dev_mm(xf, Wp, xf, g["gru_whh"][l + 1].T)
        xw, a_s, a_d = o[:, :D], o[:, D], o[:, D + 1]
        alpha = _lrelu(a_s[s] + a_d[d2], NEG_GAT)
        alpha = _seg_softmax(alpha, d2, N)
        h = _seg_sum(xw[s] * alpha[:, None], d2, N) + g["gat_b"][l]
        gi, _ = dev_mm(_elu(h), g["gru_wih"][l + 1].T)
        xf = np.maximum(_gru_gates(gi + g["gru_bih"][l + 1],
                                   gh + g["gru_bhh"][l + 1], xf), 0.0)

    # --- attentive readout ---
    out = np.maximum(_seg_sum(xf, batch, G), 0.0)
    Wp = np.concatenate([g["mol_Ws"].T,
                         (g["mol_Ws"].T @ g["mol_att_s"])[:, None]], 1)
    o, _ = dev_mm(xf, Wp)
    xs, xs_att = o[:, :D], o[:, D]
    for _ in range(3):
        xd_att = (out @ g["mol_Wd"].T) @ g["mol_att_d"]
        alpha = _lrelu(xs_att + xd_att[batch], NEG_GAT)
        alpha = _seg_softmax(alpha, batch, G)
        h = _seg_sum(xs * alpha[:, None], batch, G) + g["mol_b"]
        gi = _elu(h) @ g["mgru_wih"].T + g["mgru_bih"]
        ghm = out @ g["mgru_whh"].T + g["mgru_bhh"]
        out = np.maximum(_gru_gates(gi, ghm, out), 0.0)

    pred = out @ g["lin2_W"].T + g["lin2_b"]
    return (out.astype(f32), pred.astype(f32))
